# revision 5
# baseline (speedup 1.0000x reference)
"""Per-donor routed linear layer on 8 Trainium2 cores.

out[i] = x[i] @ W[donor_labels[i]].T + b[donor_labels[i]]

Strategy: route on host (stable sort of rows by donor label), one donor per
core, zero-padded to a common row count R.  Each core runs a dense
(R x 1024) @ (1024 x 100) matmul with K tiled 8x128, accumulating in PSUM,
bias added on the DVE during PSUM->SBUF eviction.

x is pre-permuted on host into a blocked, feature-major layout
(NB, 128 partitions, 8 k-tiles, 512 rows) so each SBUF partition's DMA read
is one contiguous 16KB run (2KB runs measured only ~20GB/s per SDMA engine;
16KB reaches line rate).  The output comes back gene-major (100, R) and is
transposed/unpermuted on host.
"""

import os
import sys

sys.path.insert(0, "/opt/trn_rl_repo")

import numpy as np

import concourse.bacc as bacc
import concourse.mybir as mybir
from concourse.tile import TileContext
from concourse.bass_utils import run_bass_kernel_spmd

N_CORES = 8
N_DONORS = 8
D_IN = 1024
N_GENES = 100
K_TILES = D_IN // 128
BLOCK = 512  # moving rows per matmul (one fp32 PSUM bank)

# "f32" = exact fp32 matmul (4 cyc/row); "f32r" = fast fp32 (~2 cyc/row,
# rel err ~1.6e-4 measured on HW)
MM_DTYPE = os.environ.get("BEC_MM_DTYPE", "f32r")


def _build_program(R: int):
    nc = bacc.Bacc(
        "TRN2",
        target_bir_lowering=False,
        debug=False,
        enable_asserts=False,
        num_devices=N_CORES,
    )
    mm_dt = mybir.dt.float32r if MM_DTYPE == "f32r" else mybir.dt.float32
    n_blocks = R // BLOCK

    xb = nc.dram_tensor(
        "xb", (n_blocks, 128, K_TILES, BLOCK), mm_dt, kind="ExternalInput"
    ).ap()
    wt = nc.dram_tensor("wt", (D_IN, N_GENES), mm_dt, kind="ExternalInput").ap()
    bias = nc.dram_tensor(
        "bias", (N_GENES, 1), mybir.dt.float32, kind="ExternalInput"
    ).ap()
    outt = nc.dram_tensor(
        "outt", (N_GENES, R), mybir.dt.float32, kind="ExternalOutput"
    ).ap()

    wt_v = wt.rearrange("(k p) n -> p k n", p=128)  # (128, K_TILES, N_GENES)

    with TileContext(nc) as tc:
        with (
            tc.tile_pool(name="const", bufs=1) as const_pool,
            tc.tile_pool(name="xp", bufs=8) as x_pool,
            tc.tile_pool(name="op", bufs=8) as out_pool,
            tc.tile_pool(name="ps", bufs=8, space="PSUM") as psum_pool,
        ):
            w_tile = const_pool.tile([128, K_TILES, N_GENES], mm_dt)
            nc.sync.dma_start(out=w_tile[:], in_=wt_v[:])
            b_tile = const_pool.tile([N_GENES, 1], mybir.dt.float32)
            nc.sync.dma_start(out=b_tile[:], in_=bias[:])

            for j in range(n_blocks):
                x_tile = x_pool.tile([128, K_TILES, BLOCK], mm_dt)
                nc.sync.dma_start(out=x_tile[:], in_=xb[j])
                psum = psum_pool.tile([N_GENES, BLOCK], mybir.dt.float32)
                for k in range(K_TILES):
                    nc.tensor.matmul(
                        out=psum[:],
                        lhsT=w_tile[:, k, :],
                        rhs=x_tile[:, k, :],
                        start=(k == 0),
                        stop=(k == K_TILES - 1),
                    )
                o_tile = out_pool.tile([N_GENES, BLOCK], mybir.dt.float32)
                nc.vector.tensor_scalar_add(
                    out=o_tile[:], in0=psum[:], scalar1=b_tile[:]
                )
                nc.sync.dma_start(
                    out=outt[:, j * BLOCK : (j + 1) * BLOCK], in_=o_tile[:]
                )

    nc.compile()
    return nc


def kernel(x, donor_labels, W, b):
    x = np.ascontiguousarray(x, dtype=np.float32)
    labels = np.asarray(donor_labels).astype(np.int64)
    W = np.asarray(W, dtype=np.float32)
    b = np.asarray(b, dtype=np.float32)
    B = x.shape[0]

    order = np.argsort(labels, kind="stable")
    counts = np.bincount(labels, minlength=N_DONORS)
    starts = np.zeros(N_DONORS + 1, dtype=np.int64)
    np.cumsum(counts, out=starts[1:])
    R = max(BLOCK, int(-(-counts.max() // BLOCK)) * BLOCK)
    n_blocks = R // BLOCK

    in_maps = []
    idx_per_core = []
    for d in range(N_CORES):
        idx = order[starts[d] : starts[d + 1]]
        idx_per_core.append(idx)
        xr = np.zeros((R, D_IN), dtype=np.float32)
        xr[: len(idx)] = x[idx]
        # (j*512+r, k*128+p) -> (j, p, k, r): one contiguous 16KB run per
        # partition per block on the device side.
        xb = np.ascontiguousarray(
            xr.reshape(n_blocks, BLOCK, K_TILES, 128).transpose(0, 3, 2, 1)
        )
        in_maps.append(
            {
                "xb": xb,
                "wt": np.ascontiguousarray(W[d].T),
                "bias": np.ascontiguousarray(b[d].reshape(N_GENES, 1)),
            }
        )

    nc = _build_program(R)
    res = run_bass_kernel_spmd(nc, in_maps, core_ids=list(range(N_CORES)))

    out = np.empty((B, N_GENES), dtype=np.float32)
    for d in range(N_CORES):
        idx = idx_per_core[d]
        out[idx] = res.results[d]["outt"][:, : len(idx)].T
    return out


# revision 6
# speedup vs baseline: 1.0800x; 1.0800x over previous
"""Per-donor routed linear layer on 8 Trainium2 cores.

out[i] = x[i] @ W[donor_labels[i]].T + b[donor_labels[i]]

Strategy: route on host (stable sort of rows by donor label), one donor per
core, zero-padded to a common row count R.  Each core runs a dense
(R x 1024) @ (1024 x 100) matmul with K tiled 8x128, accumulating in PSUM,
bias added on the DVE during PSUM->SBUF eviction.

x is pre-permuted on host into a blocked, feature-major layout
(NB, 128 partitions, 8 k-tiles, 512 rows) so each SBUF partition's DMA read
is one contiguous 16KB run (2KB runs measured only ~20GB/s per SDMA engine;
16KB reaches line rate).  The output comes back gene-major (100, R) and is
transposed/unpermuted on host.
"""

import os
import sys

sys.path.insert(0, "/opt/trn_rl_repo")

import numpy as np

import concourse.bacc as bacc
import concourse.mybir as mybir
from concourse.tile import TileContext
from concourse.bass_utils import run_bass_kernel_spmd

N_CORES = 8
N_DONORS = 8
D_IN = 1024
N_GENES = 100
K_TILES = D_IN // 128
BLOCK = 512  # moving rows per matmul (one fp32 PSUM bank)

# "f32" = exact fp32 matmul (4 cyc/row); "f32r" = fast fp32 (~2 cyc/row,
# rel err ~1.6e-4 measured on HW)
MM_DTYPE = os.environ.get("BEC_MM_DTYPE", "f32r")


def _build_program(R: int):
    nc = bacc.Bacc(
        "TRN2",
        target_bir_lowering=False,
        debug=False,
        enable_asserts=False,
        num_devices=N_CORES,
    )
    mm_dt = mybir.dt.float32r if MM_DTYPE == "f32r" else mybir.dt.float32
    n_blocks = R // BLOCK

    xb = nc.dram_tensor(
        "xb", (n_blocks, 128, K_TILES, BLOCK), mm_dt, kind="ExternalInput"
    ).ap()
    wt = nc.dram_tensor("wt", (D_IN, N_GENES), mm_dt, kind="ExternalInput").ap()
    bias = nc.dram_tensor(
        "bias", (N_GENES, 1), mybir.dt.float32, kind="ExternalInput"
    ).ap()
    outt = nc.dram_tensor(
        "outt", (N_GENES, R), mybir.dt.float32, kind="ExternalOutput"
    ).ap()

    wt_v = wt.rearrange("(k p) n -> p k n", p=128)  # (128, K_TILES, N_GENES)

    with TileContext(nc) as tc:
        with (
            tc.tile_pool(name="const", bufs=1) as const_pool,
            tc.tile_pool(name="xp", bufs=8) as x_pool,
            tc.tile_pool(name="op", bufs=8) as out_pool,
            tc.tile_pool(name="ps", bufs=8, space="PSUM") as psum_pool,
        ):
            w_tile = const_pool.tile([128, K_TILES, N_GENES], mm_dt)
            nc.sync.dma_start(out=w_tile[:], in_=wt_v[:])
            b_tile = const_pool.tile([N_GENES, 1], mybir.dt.float32)
            nc.sync.dma_start(out=b_tile[:], in_=bias[:])

            KH = K_TILES // 2
            for j in range(n_blocks):
                # two half-tiles: matmuls on the first half start while the
                # second half is still in flight
                xa = x_pool.tile([128, KH, BLOCK], mm_dt, tag="xa")
                nc.sync.dma_start(out=xa[:], in_=xb[j, :, :KH])
                xc = x_pool.tile([128, KH, BLOCK], mm_dt, tag="xc")
                nc.sync.dma_start(out=xc[:], in_=xb[j, :, KH:])
                psum = psum_pool.tile([N_GENES, BLOCK], mybir.dt.float32)
                for k in range(K_TILES):
                    src = xa if k < KH else xc
                    nc.tensor.matmul(
                        out=psum[:],
                        lhsT=w_tile[:, k, :],
                        rhs=src[:, k % KH, :],
                        start=(k == 0),
                        stop=(k == K_TILES - 1),
                    )
                o_tile = out_pool.tile([N_GENES, BLOCK], mybir.dt.float32)
                nc.vector.tensor_scalar_add(
                    out=o_tile[:], in0=psum[:], scalar1=b_tile[:]
                )
                nc.scalar.dma_start(
                    out=outt[:, j * BLOCK : (j + 1) * BLOCK], in_=o_tile[:]
                )

    nc.compile()
    return nc


def kernel(x, donor_labels, W, b):
    x = np.ascontiguousarray(x, dtype=np.float32)
    labels = np.asarray(donor_labels).astype(np.int64)
    W = np.asarray(W, dtype=np.float32)
    b = np.asarray(b, dtype=np.float32)
    B = x.shape[0]

    order = np.argsort(labels, kind="stable")
    counts = np.bincount(labels, minlength=N_DONORS)
    starts = np.zeros(N_DONORS + 1, dtype=np.int64)
    np.cumsum(counts, out=starts[1:])
    R = max(BLOCK, int(-(-counts.max() // BLOCK)) * BLOCK)
    n_blocks = R // BLOCK

    in_maps = []
    idx_per_core = []
    for d in range(N_CORES):
        idx = order[starts[d] : starts[d + 1]]
        idx_per_core.append(idx)
        xr = np.zeros((R, D_IN), dtype=np.float32)
        xr[: len(idx)] = x[idx]
        # (j*512+r, k*128+p) -> (j, p, k, r): one contiguous 16KB run per
        # partition per block on the device side.
        xb = np.ascontiguousarray(
            xr.reshape(n_blocks, BLOCK, K_TILES, 128).transpose(0, 3, 2, 1)
        )
        in_maps.append(
            {
                "xb": xb,
                "wt": np.ascontiguousarray(W[d].T),
                "bias": np.ascontiguousarray(b[d].reshape(N_GENES, 1)),
            }
        )

    nc = _build_program(R)
    res = run_bass_kernel_spmd(nc, in_maps, core_ids=list(range(N_CORES)))

    out = np.empty((B, N_GENES), dtype=np.float32)
    for d in range(N_CORES):
        idx = idx_per_core[d]
        out[idx] = res.results[d]["outt"][:, : len(idx)].T
    return out


# revision 7
# speedup vs baseline: 1.2247x; 1.1339x over previous
"""Per-donor routed linear layer on 8 Trainium2 cores.

out[i] = x[i] @ W[donor_labels[i]].T + b[donor_labels[i]]

Strategy: route on host (stable sort of rows by donor label), one donor per
core, zero-padded to a common row count R.  Each core runs a dense
(R x 1024) @ (1024 x 100) matmul with K tiled 8x128, accumulating in PSUM,
bias added on the DVE during PSUM->SBUF eviction.

x is pre-permuted on host into a blocked, feature-major layout
(NB, 128 partitions, 8 k-tiles, 512 rows) so each SBUF partition's DMA read
is one contiguous 16KB run (2KB runs measured only ~20GB/s per SDMA engine;
16KB reaches line rate).  The output comes back gene-major (100, R) and is
transposed/unpermuted on host.
"""

import os
import sys

sys.path.insert(0, "/opt/trn_rl_repo")

import numpy as np

import concourse.bacc as bacc
import concourse.mybir as mybir
from concourse.tile import TileContext
from concourse.bass_utils import run_bass_kernel_spmd

N_CORES = 8
N_DONORS = 8
D_IN = 1024
N_GENES = 100
K_TILES = D_IN // 128
BLOCK = 512  # moving rows per matmul (one fp32 PSUM bank)
OG = 4  # output blocks grouped per store DMA (8KB runs per partition)

# "f32" = exact fp32 matmul (4 cyc/row); "f32r" = fast fp32 (~2 cyc/row,
# rel err ~1.6e-4 measured on HW)
MM_DTYPE = os.environ.get("BEC_MM_DTYPE", "f32r")


def _build_program(R: int):
    nc = bacc.Bacc(
        "TRN2",
        target_bir_lowering=False,
        debug=False,
        enable_asserts=False,
        num_devices=N_CORES,
    )
    mm_dt = mybir.dt.float32r if MM_DTYPE == "f32r" else mybir.dt.float32
    n_blocks = R // BLOCK

    xb = nc.dram_tensor(
        "xb", (n_blocks, 128, K_TILES, BLOCK), mm_dt, kind="ExternalInput"
    ).ap()
    wb = nc.dram_tensor(
        "wb", (128, K_TILES, N_GENES), mm_dt, kind="ExternalInput"
    ).ap()
    bias = nc.dram_tensor(
        "bias", (N_GENES, 1), mybir.dt.float32, kind="ExternalInput"
    ).ap()
    outt = nc.dram_tensor(
        "outt", (N_GENES, R), mybir.dt.float32, kind="ExternalOutput"
    ).ap()

    with TileContext(nc) as tc:
        with (
            tc.tile_pool(name="const", bufs=1) as const_pool,
            tc.tile_pool(name="xp", bufs=8) as x_pool,
            tc.tile_pool(name="op", bufs=8) as out_pool,
            tc.tile_pool(name="ps", bufs=8, space="PSUM") as psum_pool,
        ):
            w_tile = const_pool.tile([128, K_TILES, N_GENES], mm_dt)
            nc.scalar.dma_start(out=w_tile[:], in_=wb[:])
            b_tile = const_pool.tile([N_GENES, 1], mybir.dt.float32)
            nc.scalar.dma_start(out=b_tile[:], in_=bias[:])

            KH = K_TILES // 2
            for j in range(n_blocks):
                # two half-tiles: matmuls on the first half start while the
                # second half is still in flight
                xa = x_pool.tile([128, KH, BLOCK], mm_dt, tag="xa")
                nc.sync.dma_start(out=xa[:], in_=xb[j, :, :KH])
                xc = x_pool.tile([128, KH, BLOCK], mm_dt, tag="xc")
                nc.sync.dma_start(out=xc[:], in_=xb[j, :, KH:])
                psum = psum_pool.tile([N_GENES, BLOCK], mybir.dt.float32)
                for k in range(K_TILES):
                    src = xa if k < KH else xc
                    nc.tensor.matmul(
                        out=psum[:],
                        lhsT=w_tile[:, k, :],
                        rhs=src[:, k % KH, :],
                        start=(k == 0),
                        stop=(k == K_TILES - 1),
                    )
                if j % OG == 0:
                    gsize = min(OG, n_blocks - j)
                    o_tile = out_pool.tile([N_GENES, OG, BLOCK], mybir.dt.float32)
                nc.vector.tensor_scalar_add(
                    out=o_tile[:, j % OG, :], in0=psum[:], scalar1=b_tile[:]
                )
                if j % OG == gsize - 1:
                    g0 = (j // OG) * OG * BLOCK
                    nc.scalar.dma_start(
                        out=outt[:, g0 : g0 + gsize * BLOCK],
                        in_=o_tile[:, :gsize, :],
                    )

    nc.compile()
    return nc


def kernel(x, donor_labels, W, b):
    x = np.ascontiguousarray(x, dtype=np.float32)
    labels = np.asarray(donor_labels).astype(np.int64)
    W = np.asarray(W, dtype=np.float32)
    b = np.asarray(b, dtype=np.float32)
    B = x.shape[0]

    order = np.argsort(labels, kind="stable")
    counts = np.bincount(labels, minlength=N_DONORS)
    starts = np.zeros(N_DONORS + 1, dtype=np.int64)
    np.cumsum(counts, out=starts[1:])
    R = max(BLOCK, int(-(-counts.max() // BLOCK)) * BLOCK)
    n_blocks = R // BLOCK

    in_maps = []
    idx_per_core = []
    for d in range(N_CORES):
        idx = order[starts[d] : starts[d + 1]]
        idx_per_core.append(idx)
        xr = np.zeros((R, D_IN), dtype=np.float32)
        xr[: len(idx)] = x[idx]
        # (j*512+r, k*128+p) -> (j, p, k, r): one contiguous 16KB run per
        # partition per block on the device side.
        xb = np.ascontiguousarray(
            xr.reshape(n_blocks, BLOCK, K_TILES, 128).transpose(0, 3, 2, 1)
        )
        in_maps.append(
            {
                "xb": xb,
                "wb": np.ascontiguousarray(
                    W[d].T.reshape(K_TILES, 128, N_GENES).transpose(1, 0, 2)
                ),
                "bias": np.ascontiguousarray(b[d].reshape(N_GENES, 1)),
            }
        )

    nc = _build_program(R)
    res = run_bass_kernel_spmd(nc, in_maps, core_ids=list(range(N_CORES)))

    out = np.empty((B, N_GENES), dtype=np.float32)
    for d in range(N_CORES):
        idx = idx_per_core[d]
        out[idx] = res.results[d]["outt"][:, : len(idx)].T
    return out


# revision 8
# speedup vs baseline: 1.8907x; 1.5438x over previous
"""Per-donor routed linear layer on 8 Trainium2 cores.

out[i] = x[i] @ W[donor_labels[i]].T + b[donor_labels[i]]

Strategy: route on host (stable sort of rows by donor label), one donor per
core, zero-padded to a common row count R.  Each core runs a dense
(R x 1024) @ (1024 x 100) matmul with K tiled 8x128, accumulating in PSUM,
bias added on the DVE during PSUM->SBUF eviction.

x is pre-permuted on host into a blocked, feature-major layout
(NB, 128 partitions, 8 k-tiles, 512 rows) so each SBUF partition's DMA read
is one contiguous 16KB run (2KB runs measured only ~20GB/s per SDMA engine;
16KB reaches line rate).  The output comes back gene-major (100, R) and is
transposed/unpermuted on host.
"""

import os
import sys

sys.path.insert(0, "/opt/trn_rl_repo")

import numpy as np

import concourse.bacc as bacc
import concourse.mybir as mybir
from concourse.tile import TileContext
from concourse.bass_utils import run_bass_kernel_spmd

N_CORES = 8
N_DONORS = 8
D_IN = 1024
N_GENES = 100
K_TILES = D_IN // 128
BLOCK = 512  # moving rows per matmul (one fp32 PSUM bank)
OG = 4  # output blocks grouped per store DMA (8KB runs per partition)

# "f32"  = exact fp32 matmul (4 cyc/row, rel err ~9e-7)
# "f32r" = fast fp32 (~2.5 cyc/row effective, rel err ~1.6e-4 measured)
# "f16"  = fp16 operands (1 cyc/row, half DMA traffic; 10-bit mantissa is the
#          same class as f32r's internal rounding)
MM_DTYPE = os.environ.get("BEC_MM_DTYPE", "f16")
NP_IN_DT = {"f32": np.float32, "f32r": np.float32, "f16": np.float16}[MM_DTYPE]


def _build_program(R: int):
    nc = bacc.Bacc(
        "TRN2",
        target_bir_lowering=False,
        debug=False,
        enable_asserts=False,
        num_devices=N_CORES,
    )
    mm_dt = {
        "f32": mybir.dt.float32,
        "f32r": mybir.dt.float32r,
        "f16": mybir.dt.float16,
    }[MM_DTYPE]
    n_blocks = R // BLOCK

    xb = nc.dram_tensor(
        "xb", (n_blocks, 128, K_TILES, BLOCK), mm_dt, kind="ExternalInput"
    ).ap()
    wb = nc.dram_tensor(
        "wb", (128, K_TILES, N_GENES), mm_dt, kind="ExternalInput"
    ).ap()
    bias = nc.dram_tensor(
        "bias", (N_GENES, 1), mybir.dt.float32, kind="ExternalInput"
    ).ap()
    outt = nc.dram_tensor(
        "outt", (N_GENES, R), mybir.dt.float32, kind="ExternalOutput"
    ).ap()

    with TileContext(nc) as tc:
        with (
            tc.tile_pool(name="const", bufs=1) as const_pool,
            tc.tile_pool(name="xp", bufs=8) as x_pool,
            tc.tile_pool(name="op", bufs=8) as out_pool,
            tc.tile_pool(name="ps", bufs=8, space="PSUM") as psum_pool,
        ):
            w_tile = const_pool.tile([128, K_TILES, N_GENES], mm_dt)
            nc.scalar.dma_start(out=w_tile[:], in_=wb[:])
            b_tile = const_pool.tile([N_GENES, 1], mybir.dt.float32)
            nc.scalar.dma_start(out=b_tile[:], in_=bias[:])

            KH = K_TILES // 2
            for j in range(n_blocks):
                # two half-tiles: matmuls on the first half start while the
                # second half is still in flight
                xa = x_pool.tile([128, KH, BLOCK], mm_dt, tag="xa")
                nc.sync.dma_start(out=xa[:], in_=xb[j, :, :KH])
                xc = x_pool.tile([128, KH, BLOCK], mm_dt, tag="xc")
                nc.sync.dma_start(out=xc[:], in_=xb[j, :, KH:])
                psum = psum_pool.tile([N_GENES, BLOCK], mybir.dt.float32)
                for k in range(K_TILES):
                    src = xa if k < KH else xc
                    nc.tensor.matmul(
                        out=psum[:],
                        lhsT=w_tile[:, k, :],
                        rhs=src[:, k % KH, :],
                        start=(k == 0),
                        stop=(k == K_TILES - 1),
                    )
                if j % OG == 0:
                    gsize = min(OG, n_blocks - j)
                    o_tile = out_pool.tile([N_GENES, OG, BLOCK], mybir.dt.float32)
                nc.vector.tensor_scalar_add(
                    out=o_tile[:, j % OG, :], in0=psum[:], scalar1=b_tile[:]
                )
                if j % OG == gsize - 1:
                    g0 = (j // OG) * OG * BLOCK
                    nc.scalar.dma_start(
                        out=outt[:, g0 : g0 + gsize * BLOCK],
                        in_=o_tile[:, :gsize, :],
                    )

    nc.compile()
    return nc


def kernel(x, donor_labels, W, b):
    x = np.ascontiguousarray(x, dtype=np.float32)
    labels = np.asarray(donor_labels).astype(np.int64)
    W = np.asarray(W, dtype=np.float32)
    b = np.asarray(b, dtype=np.float32)
    B = x.shape[0]

    order = np.argsort(labels, kind="stable")
    counts = np.bincount(labels, minlength=N_DONORS)
    starts = np.zeros(N_DONORS + 1, dtype=np.int64)
    np.cumsum(counts, out=starts[1:])
    R = max(BLOCK, int(-(-counts.max() // BLOCK)) * BLOCK)
    n_blocks = R // BLOCK

    in_maps = []
    idx_per_core = []
    for d in range(N_CORES):
        idx = order[starts[d] : starts[d + 1]]
        idx_per_core.append(idx)
        xr = np.zeros((R, D_IN), dtype=NP_IN_DT)
        xr[: len(idx)] = x[idx].astype(NP_IN_DT)
        # (j*512+r, k*128+p) -> (j, p, k, r): one contiguous 16KB run per
        # partition per block on the device side.
        xb = np.ascontiguousarray(
            xr.reshape(n_blocks, BLOCK, K_TILES, 128).transpose(0, 3, 2, 1)
        )
        in_maps.append(
            {
                "xb": xb,
                "wb": np.ascontiguousarray(
                    W[d].T.reshape(K_TILES, 128, N_GENES).transpose(1, 0, 2)
                ).astype(NP_IN_DT),
                "bias": np.ascontiguousarray(b[d].reshape(N_GENES, 1)),
            }
        )

    nc = _build_program(R)
    res = run_bass_kernel_spmd(nc, in_maps, core_ids=list(range(N_CORES)))

    out = np.empty((B, N_GENES), dtype=np.float32)
    for d in range(N_CORES):
        idx = idx_per_core[d]
        out[idx] = res.results[d]["outt"][:, : len(idx)].T
    return out


# revision 10
# speedup vs baseline: 2.0533x; 1.0860x over previous
"""Per-donor routed linear layer on 8 Trainium2 cores.

out[i] = x[i] @ W[donor_labels[i]].T + b[donor_labels[i]]

Strategy: route on host (stable sort of rows by donor label), one donor per
core, zero-padded to a common row count R.  Each core runs a dense
(R x 1024) @ (1024 x 100) matmul with K tiled 8x128, accumulating in PSUM,
bias added on the DVE during PSUM->SBUF eviction.

x is pre-permuted on host into a blocked, feature-major layout
(NB, 128 partitions, 8 k-tiles, 512 rows) so each SBUF partition's DMA read
is one contiguous 16KB run (2KB runs measured only ~20GB/s per SDMA engine;
16KB reaches line rate).  The output comes back gene-major (100, R) and is
transposed/unpermuted on host.
"""

import os
import sys

sys.path.insert(0, "/opt/trn_rl_repo")

import numpy as np

import concourse.bacc as bacc
import concourse.mybir as mybir
from concourse.tile import TileContext
from concourse.bass_utils import run_bass_kernel_spmd

N_CORES = 8
N_DONORS = 8
D_IN = 1024
N_GENES = 100
K_TILES = D_IN // 128
BLOCK = 512  # moving rows per matmul (one fp32 PSUM bank)
OG = 8  # output blocks grouped per store DMA (16KB runs per partition)

# "f32"  = exact fp32 matmul (4 cyc/row, rel err ~9e-7)
# "f32r" = fast fp32 (~2.5 cyc/row effective, rel err ~1.6e-4 measured)
# "f16"  = fp16 operands (1 cyc/row, half DMA traffic; 10-bit mantissa is the
#          same class as f32r's internal rounding)
MM_DTYPE = os.environ.get("BEC_MM_DTYPE", "f16")
NP_IN_DT = {"f32": np.float32, "f32r": np.float32, "f16": np.float16}[MM_DTYPE]


def _build_program(R: int):
    nc = bacc.Bacc(
        "TRN2",
        target_bir_lowering=False,
        debug=False,
        enable_asserts=False,
        num_devices=N_CORES,
    )
    mm_dt = {
        "f32": mybir.dt.float32,
        "f32r": mybir.dt.float32r,
        "f16": mybir.dt.float16,
    }[MM_DTYPE]
    n_blocks = R // BLOCK

    xb = nc.dram_tensor(
        "xb", (n_blocks, 128, K_TILES, BLOCK), mm_dt, kind="ExternalInput"
    ).ap()
    wb = nc.dram_tensor(
        "wb", (128, K_TILES, N_GENES), mm_dt, kind="ExternalInput"
    ).ap()
    bias = nc.dram_tensor(
        "bias", (N_GENES, 1), mybir.dt.float32, kind="ExternalInput"
    ).ap()
    outt = nc.dram_tensor(
        "outt", (N_GENES, R), mybir.dt.float32, kind="ExternalOutput"
    ).ap()

    with TileContext(nc) as tc:
        with (
            tc.tile_pool(name="const", bufs=1) as const_pool,
            tc.tile_pool(name="xp", bufs=12) as x_pool,
            tc.tile_pool(name="op", bufs=3) as out_pool,
            tc.tile_pool(name="ps", bufs=8, space="PSUM") as psum_pool,
        ):
            w_tile = const_pool.tile([128, K_TILES, N_GENES], mm_dt)
            nc.scalar.dma_start(out=w_tile[:], in_=wb[:])
            b_tile = const_pool.tile([N_GENES, 1], mybir.dt.float32)
            nc.scalar.dma_start(out=b_tile[:], in_=bias[:])

            for j in range(n_blocks):
                x_tile = x_pool.tile([128, K_TILES, BLOCK], mm_dt)
                nc.sync.dma_start(out=x_tile[:], in_=xb[j])
                psum = psum_pool.tile([N_GENES, BLOCK], mybir.dt.float32)
                for k in range(K_TILES):
                    nc.tensor.matmul(
                        out=psum[:],
                        lhsT=w_tile[:, k, :],
                        rhs=x_tile[:, k, :],
                        start=(k == 0),
                        stop=(k == K_TILES - 1),
                    )
                if j % OG == 0:
                    gsize = min(OG, n_blocks - j)
                    o_tile = out_pool.tile([N_GENES, OG, BLOCK], mybir.dt.float32)
                nc.vector.tensor_scalar_add(
                    out=o_tile[:, j % OG, :], in0=psum[:], scalar1=b_tile[:]
                )
                if j % OG == gsize - 1:
                    g0 = (j // OG) * OG * BLOCK
                    nc.scalar.dma_start(
                        out=outt[:, g0 : g0 + gsize * BLOCK],
                        in_=o_tile[:, :gsize, :],
                    )

    nc.compile()
    return nc


def kernel(x, donor_labels, W, b):
    x = np.ascontiguousarray(x, dtype=np.float32)
    labels = np.asarray(donor_labels).astype(np.int64)
    W = np.asarray(W, dtype=np.float32)
    b = np.asarray(b, dtype=np.float32)
    B = x.shape[0]

    order = np.argsort(labels, kind="stable")
    counts = np.bincount(labels, minlength=N_DONORS)
    starts = np.zeros(N_DONORS + 1, dtype=np.int64)
    np.cumsum(counts, out=starts[1:])
    R = max(BLOCK, int(-(-counts.max() // BLOCK)) * BLOCK)
    n_blocks = R // BLOCK

    in_maps = []
    idx_per_core = []
    for d in range(N_CORES):
        idx = order[starts[d] : starts[d + 1]]
        idx_per_core.append(idx)
        xr = np.zeros((R, D_IN), dtype=NP_IN_DT)
        xr[: len(idx)] = x[idx].astype(NP_IN_DT)
        # (j*512+r, k*128+p) -> (j, p, k, r): one contiguous 16KB run per
        # partition per block on the device side.
        xb = np.ascontiguousarray(
            xr.reshape(n_blocks, BLOCK, K_TILES, 128).transpose(0, 3, 2, 1)
        )
        in_maps.append(
            {
                "xb": xb,
                "wb": np.ascontiguousarray(
                    W[d].T.reshape(K_TILES, 128, N_GENES).transpose(1, 0, 2)
                ).astype(NP_IN_DT),
                "bias": np.ascontiguousarray(b[d].reshape(N_GENES, 1)),
            }
        )

    nc = _build_program(R)
    res = run_bass_kernel_spmd(nc, in_maps, core_ids=list(range(N_CORES)))

    out = np.empty((B, N_GENES), dtype=np.float32)
    for d in range(N_CORES):
        idx = idx_per_core[d]
        out[idx] = res.results[d]["outt"][:, : len(idx)].T
    return out
